# revision 5
# baseline (speedup 1.0000x reference)
"""Trainium2 Bass kernel for MaskedPiecewiseRationalQuadraticAutoregressiveTransform.

Self-contained: hardcodes all shapes. Strategy:
 - Pure data parallel over 8 cores (batch 65536 -> 8192/core).
 - MADE network computed in transposed-activation layout [hidden, batch]:
   masked weights precomputed on host, hidden units degree-sorted so
   mask-zero 128x128 blocks are skipped (10/16 matmuls per layer).
   Final layer swaps stationary/moving operands so params land as
   [batch, outputs] in PSUM directly (no transpose needed).
 - Rational-quadratic spline evaluated on DVE/ACT engines with:
   segmented cumsum via tensor_tensor_scan, edge computation via
   scalar_tensor_tensor with a constant bias tile, searchsorted via 9
   broadcast compares, and a monotone copy_predicated cascade that
   gathers all 6 per-bin values (edge_w/edge_h/deriv lo+hi) at once.
"""
import numpy as np

import concourse.bass as bass
import concourse.tile as tile
from concourse import bacc, mybir
from concourse.bass_utils import run_bass_kernel_spmd

F32 = mybir.dt.float32
F32R = mybir.dt.float32r
U8 = mybir.dt.uint8
AL = mybir.AluOpType
AF = mybir.ActivationFunctionType

FEAT = 64
HID = 512
NB = 10           # num bins
MULT = 3 * NB - 1  # 29
TAIL = 3.0
MIN_W = 1e-3
MIN_H = 1e-3
MIN_D = 1e-3
NBLK = 2
B_FULL = 65536
NCORES = 8
BSH = B_FULL // NCORES   # 8192 per core
NBT = 512                # samples per b-tile
KC = HID // 128          # 4 hidden chunks
NCH = 4                  # final-layer output column chunks (464 wide each)
OW = 464                 # 29*16 outputs per column chunk
SCALE = 1.0 / np.sqrt(HID)
C1W = 1.0 - MIN_W * NB
C1H = 1.0 - MIN_H * NB

# lazily built & cached compiled module
_CACHE = {}


def _host_prep(W0, b0, Wr, br, Wf, bf):
    """Mask, degree-sort, and reorder weights; returns device tensors."""
    in_deg = np.arange(1, FEAT + 1)
    hid_deg = np.arange(HID) % (FEAT - 1) + 1
    m0 = (hid_deg[:, None] >= in_deg[None, :]).astype(np.float32)
    mh = (hid_deg[:, None] >= hid_deg[None, :]).astype(np.float32)
    out_deg = np.repeat(in_deg, MULT)
    mf = (out_deg[:, None] > hid_deg[None, :]).astype(np.float32)

    perm = np.argsort(hid_deg, kind='stable')
    W0m = (m0 * W0)[perm, :]                        # [H, F]
    b0s = b0[perm]
    Wrm = (mh * Wr.reshape(-1, HID, HID)).reshape(Wr.shape)
    Wrm = Wrm[:, :, perm, :][:, :, :, perm]
    brs = br[:, :, perm]
    Wfm = (mf * Wf)[:, perm]                        # [O, H]

    # final-layer row reorder, PSUM-bank-group-major: group g = features
    # 16g..16g+15 -> rows g*464 + [uw(160) | uh(160) | ud(144)], f-major
    # k-contiguous within each section.
    row_perm = np.zeros(FEAT * MULT, dtype=np.int64)
    for g in range(4):
        for fi in range(16):
            f = 16 * g + fi
            for k in range(NB):
                row_perm[g * 464 + fi * NB + k] = f * MULT + k
                row_perm[g * 464 + 160 + fi * NB + k] = f * MULT + NB + k
            for k in range(NB - 1):
                row_perm[g * 464 + 320 + fi * 9 + k] = f * MULT + 2 * NB + k
    Wfo = Wfm[row_perm, :]
    bfo = bf[row_perm]

    dev = {
        "lhsT0": np.ascontiguousarray(W0m.T),                 # [64, 512]
        "wrT": np.ascontiguousarray(
            np.transpose(Wrm, (0, 1, 3, 2))),                 # [2,2,Hin,Hout]
        "wfT": np.ascontiguousarray(Wfo.T),                   # [512, 1856]
        "bias_relu0": b0s,
        "bias_relu1": b0s + brs[0, 1],
        "bias_tfin": b0s + brs[0, 1] + brs[1, 1],
        "bias_r1_0": brs[0, 0],
        "bias_r1_1": brs[1, 0],
        "bf_out": bfo,
    }
    return dev


def _consts():
    ident = np.eye(128, dtype=np.float32)
    maskseg = np.tile(np.array([0.0] + [1.0] * (NB - 1), np.float32),
                      (128, FEAT))                            # [128, 640]
    j = np.arange(1, NB, dtype=np.float32)
    b9w = np.tile((6.0 * MIN_W * j - TAIL), (128, FEAT))      # [128, 576]
    b9h = np.tile((6.0 * MIN_H * j - TAIL), (128, FEAT))
    return {"ident": ident, "maskseg": maskseg, "bias9w": b9w, "bias9h": b9h}


def build(n_tiles=BSH // NBT, mm_f32r=True, has_bf=False):
    nc = bacc.Bacc(None, target_bir_lowering=False, debug=False)
    nbsh = n_tiles * NBT
    MDT = F32R if mm_f32r else F32

    # --- dram I/O ---
    xs = nc.dram_tensor("xs", [nbsh, FEAT], F32, kind="ExternalInput")
    lhsT0 = nc.dram_tensor("lhsT0", [FEAT, HID], MDT, kind="ExternalInput")
    wrT = nc.dram_tensor("wrT", [NBLK, 2, HID, HID], MDT, kind="ExternalInput")
    wfT = nc.dram_tensor("wfT", [HID, FEAT * MULT], MDT, kind="ExternalInput")
    b_relu0 = nc.dram_tensor("bias_relu0", [HID], F32, kind="ExternalInput")
    b_relu1 = nc.dram_tensor("bias_relu1", [HID], F32, kind="ExternalInput")
    b_tfin = nc.dram_tensor("bias_tfin", [HID], F32, kind="ExternalInput")
    b_r1_0 = nc.dram_tensor("bias_r1_0", [HID], F32, kind="ExternalInput")
    b_r1_1 = nc.dram_tensor("bias_r1_1", [HID], F32, kind="ExternalInput")
    bf_d = nc.dram_tensor("bf_out", [FEAT * MULT], MDT, kind="ExternalInput")
    ident_d = nc.dram_tensor("ident", [128, 128], F32, kind="ExternalInput")
    maskseg_d = nc.dram_tensor("maskseg", [128, 640], F32, kind="ExternalInput")
    b9w_d = nc.dram_tensor("bias9w", [128, 576], F32, kind="ExternalInput")
    b9h_d = nc.dram_tensor("bias9h", [128, 576], F32, kind="ExternalInput")
    y = nc.dram_tensor("y", [nbsh, FEAT], F32, kind="ExternalOutput")

    x_v = xs[:].rearrange("(bt c p) f -> bt p c f", p=128, c=4)
    y_v = y[:].rearrange("(bt c p) f -> bt p c f", p=128, c=4)


    with tile.TileContext(nc) as tc:
        with (
            tc.tile_pool(name="wpool", bufs=1) as wp,
            tc.tile_pool(name="xpool", bufs=2) as xp,
            tc.tile_pool(name="rlu", bufs=2) as rp,
            tc.tile_pool(name="tsbp", bufs=2) as tp,
            tc.tile_pool(name="spl", bufs=2) as sp,
            tc.tile_pool(name="sm", bufs=3) as smp,
            tc.tile_pool(name="yp", bufs=2) as yp,
            tc.tile_pool(name="psA", bufs=1, space="PSUM") as psA,
            tc.tile_pool(name="psB", bufs=1, space="PSUM") as psB,
        ):
            # ---- load weights/constants (resident) ----
            w0_sb = wp.tile([FEAT, HID], MDT, tag="w0")
            nc.sync.dma_start(w0_sb[:], lhsT0[:])
            wr_sb = {}
            for i in range(NBLK):
                for jj in range(2):
                    t = wp.tile([128, KC, HID], MDT, tag=f"wr{i}{jj}")
                    nc.sync.dma_start(
                        t[:], wrT[i, jj].rearrange("(kc p) m -> p kc m", p=128))
                    wr_sb[(i, jj)] = t
            wf_sb = wp.tile([128, KC, FEAT * MULT], MDT, tag="wf")
            nc.sync.dma_start(
                wf_sb[:], wfT[:].rearrange("(kc p) m -> p kc m", p=128))
            bcols = {}
            for nm, dt_ in [("r0", b_relu0), ("r1", b_relu1), ("tf", b_tfin),
                            ("b10", b_r1_0), ("b11", b_r1_1)]:
                t = wp.tile([128, KC], F32, tag=f"b{nm}")
                nc.sync.dma_start(t[:], dt_[:].rearrange("(c p) -> p c", p=128))
                bcols[nm] = t
            ident_sb = wp.tile([128, 128], F32, tag="ident")
            nc.sync.dma_start(ident_sb[:], ident_d[:])
            mseg_sb = wp.tile([128, 640], F32, tag="mseg")
            nc.sync.dma_start(mseg_sb[:], maskseg_d[:])
            b9w_sb = wp.tile([128, 576], F32, tag="b9w")
            nc.sync.dma_start(b9w_sb[:], b9w_d[:])
            b9h_sb = wp.tile([128, 576], F32, tag="b9h")
            nc.sync.dma_start(b9h_sb[:], b9h_d[:])
            if has_bf:
                bf_sb = wp.tile([1, FEAT * MULT], MDT, tag="bfr")
                nc.sync.dma_start(bf_sb[:], bf_d[0:FEAT * MULT].unsqueeze(0))
                ones_sb = wp.tile([1, 128], MDT, tag="ones")
                nc.vector.memset(ones_sb[:], 1.0)

            for bt in range(n_tiles):
                # ---- load x tile & transpose ----
                x_sb = xp.tile([128, 4, FEAT], F32, tag="x")
                nc.sync.dma_start(x_sb[:], x_v[bt])
                xT_ps = psA.tile([64, NBT], F32, tag="t")
                for c in range(4):
                    nc.tensor.transpose(
                        xT_ps[:, c * 128:(c + 1) * 128], x_sb[:, c, :],
                        ident_sb[:])
                xT_sb = xp.tile([64, NBT], MDT, tag="xT")
                nc.scalar.copy(xT_sb[:], xT_ps[:])

                # ---- layer0: t[mc] = W0m[mc] @ xT ----
                t_ps = psA.tile([128, KC, NBT], F32, tag="t")
                for mc in range(KC):
                    nc.tensor.matmul(
                        t_ps[:, mc, :],
                        (w0_sb[:, mc * 128:(mc + 1) * 128]),
                        (xT_sb[:]),
                        start=True, stop=False)

                # ---- residual blocks ----
                relu_in = rp.tile([128, KC, NBT], MDT, tag="relu_in")
                for mc in range(KC):
                    nc.scalar.activation(
                        relu_in[:, mc, :], t_ps[:, mc, :], AF.Relu,
                        bias=bcols["r0"][:, mc:mc + 1], scale=1.0)
                for i in range(NBLK):
                    r1_ps = psB.tile([128, KC, NBT], F32, tag="r")
                    for mc in range(KC):
                        for ki, kc in enumerate(range(mc + 1)):
                            nc.tensor.matmul(
                                r1_ps[:, mc, :],
                                (wr_sb[(i, 0)][:, kc,
                                                   mc * 128:(mc + 1) * 128]),
                                (relu_in[:, kc, :]),
                                start=(ki == 0), stop=(kc == mc))
                    relu_mid = rp.tile([128, KC, NBT], MDT, tag="relu_mid")
                    bkey = "b10" if i == 0 else "b11"
                    for mc in range(KC):
                        nc.scalar.activation(
                            relu_mid[:, mc, :], r1_ps[:, mc, :], AF.Relu,
                            bias=bcols[bkey][:, mc:mc + 1], scale=1.0)
                    # r2 accumulates into t_ps
                    last = (i == NBLK - 1)
                    for mc in range(KC):
                        for kc in range(mc + 1):
                            nc.tensor.matmul(
                                t_ps[:, mc, :],
                                (wr_sb[(i, 1)][:, kc,
                                                   mc * 128:(mc + 1) * 128]),
                                (relu_mid[:, kc, :]),
                                start=False, stop=(last and kc == mc))
                    if not last:
                        relu_in = rp.tile([128, KC, NBT], MDT, tag="relu_in")
                        for mc in range(KC):
                            nc.scalar.activation(
                                relu_in[:, mc, :], t_ps[:, mc, :], AF.Relu,
                                bias=bcols["r1"][:, mc:mc + 1], scale=1.0)

                # ---- t -> SBUF (with final bias); frees psA for next tile
                t_sb = tp.tile([128, KC, NBT], MDT, tag="tsb")
                for mc in range(KC):
                    nc.scalar.activation(
                        t_sb[:, mc, :], t_ps[:, mc, :], AF.Identity,
                        bias=bcols["tf"][:, mc:mc + 1], scale=1.0)

                ystage = yp.tile([128, 4, FEAT], F32, tag="y")

                # ---- final layer + spline, per 128-sample chunk ----
                for c in range(4):
                    p_ps = psB.tile([128, NCH, 512], F32, tag="r")
                    for nch in range(NCH):
                        nkc = nch + 1  # triangular: kc <= nch
                        for kc in range(nkc):
                            nc.tensor.matmul(
                                p_ps[:, nch, 0:OW],
                                (t_sb[:, kc, c * 128:(c + 1) * 128]),
                                (wf_sb[:, kc, nch * OW:(nch + 1) * OW]),
                                start=(kc == 0),
                                stop=(kc == nkc - 1 and not has_bf))
                        if has_bf:
                            nc.tensor.matmul(
                                p_ps[:, nch, 0:OW],
                                (ones_sb[:, 0:128]),
                                (bf_sb[:, nch * OW:(nch + 1) * OW]),
                                start=False, stop=True)
                    _spline(nc, sp, smp, p_ps, x_sb, ystage, c,
                            mseg_sb, b9w_sb, b9h_sb)

                nc.sync.dma_start(y_v[bt], ystage[:])

    nc.compile()
    return nc


def _spline(nc, sp, smp, p_ps, x_sb, ystage, c, mseg_sb, b9w_sb, b9h_sb):
    """Spline for one 128-sample chunk. p_ps: [128, NCH, OW] psum with
    param layout uw(640 f-major k), uh(640), ud(576 f-major 9/f)."""
    P = 128

    # params layout: [p, group(4, bank-aligned 512), uw(160)|uh(160)|ud(144)]
    EW = sp.tile([P, 640], F32, tag="EW")
    nc.scalar.activation(
        EW[:].rearrange("p (g a) -> p g a", g=4), p_ps[:, :, 0:160],
        AF.Exp, scale=SCALE)
    EH = sp.tile([P, 640], F32, tag="EH")
    nc.scalar.activation(
        EH[:].rearrange("p (g a) -> p g a", g=4), p_ps[:, :, 160:320],
        AF.Exp, scale=SCALE)
    ESP = sp.tile([P, 576], F32, tag="ESP")
    nc.scalar.activation(
        ESP[:].rearrange("p (g a) -> p g a", g=4), p_ps[:, :, 320:464],
        AF.Exp, scale=1.0)
    DSP = sp.tile([P, 576], F32, tag="DSP")
    nc.scalar.activation(DSP[:], ESP[:], AF.Ln, bias=1.0, scale=1.0)

    # segmented inclusive cumsums
    SW = sp.tile([P, 640], F32, tag="SW")
    nc.vector.tensor_tensor_scan(SW[:], mseg_sb[:], EW[:], 0.0,
                                 op0=AL.mult, op1=AL.add)
    SH = sp.tile([P, 640], F32, tag="SH")
    nc.vector.tensor_tensor_scan(SH[:], mseg_sb[:], EH[:], 0.0,
                                 op0=AL.mult, op1=AL.add)
    SWk = SW[:].rearrange("p (f k) -> p f k", k=NB)
    SHk = SH[:].rearrange("p (f k) -> p f k", k=NB)

    rW = sp.tile([P, FEAT], F32, tag="rW")
    nc.vector.reciprocal(rW[:], SWk[:, :, NB - 1:NB].squeeze(2))
    rH = sp.tile([P, FEAT], F32, tag="rH")
    nc.vector.reciprocal(rH[:], SHk[:, :, NB - 1:NB].squeeze(2))

    AW = sp.tile([P, 640], F32, tag="AW")
    nc.vector.tensor_tensor(
        out=AW[:].rearrange("p (f k) -> p f k", k=NB), in0=SWk,
        in1=rW[:].unsqueeze(2).to_broadcast([P, FEAT, NB]), op=AL.mult)
    AH = sp.tile([P, 640], F32, tag="AH")
    nc.vector.tensor_tensor(
        out=AH[:].rearrange("p (f k) -> p f k", k=NB), in0=SHk,
        in1=rH[:].unsqueeze(2).to_broadcast([P, FEAT, NB]), op=AL.mult)

    # G table: [128, f(64), k(11), 3] = (edge_w[k], edge_h[k], d[k])
    G = sp.tile([P, FEAT, 11, 3], F32, tag="G")
    AWk = AW[:].rearrange("p (f k) -> p f k", k=NB)
    AHk = AH[:].rearrange("p (f k) -> p f k", k=NB)
    nc.vector.scalar_tensor_tensor(
        out=G[:, :, 1:10, 0], in0=AWk[:, :, 0:9], scalar=6.0 * C1W,
        in1=b9w_sb[:].rearrange("p (f j) -> p f j", j=9),
        op0=AL.mult, op1=AL.add)
    nc.vector.scalar_tensor_tensor(
        out=G[:, :, 1:10, 1], in0=AHk[:, :, 0:9], scalar=6.0 * C1H,
        in1=b9h_sb[:].rearrange("p (f j) -> p f j", j=9),
        op0=AL.mult, op1=AL.add)
    nc.vector.tensor_scalar_add(
        G[:, :, 1:10, 2], DSP[:].rearrange("p (f j) -> p f j", j=9), MIN_D)
    # boundaries
    nc.gpsimd.memset(G[:, :, 0, 0], -TAIL)
    nc.gpsimd.memset(G[:, :, 0, 1], -TAIL)
    nc.gpsimd.memset(G[:, :, 0, 2], 1.0)
    nc.gpsimd.memset(G[:, :, 10, 0], TAIL)
    nc.gpsimd.memset(G[:, :, 10, 1], TAIL)
    nc.gpsimd.memset(G[:, :, 10, 2], 1.0)

    # xc = clip(x, -3, 3)
    xv = x_sb[:, c, :]
    xc = smp.tile([P, FEAT], F32, tag="xc")
    nc.vector.tensor_scalar(out=xc[:], in0=xv, scalar1=-TAIL, scalar2=TAIL,
                            op0=AL.max, op1=AL.min)

    # searchsorted preds: PRED[:, f, j] = (xc >= edge_w[j+1]), j=0..8
    PRED = sp.tile([P, FEAT, 9], U8, tag="PRED")
    nc.vector.tensor_tensor(
        out=PRED[:], in0=xc[:].unsqueeze(2).to_broadcast([P, FEAT, 9]),
        in1=G[:, :, 1:10, 0], op=AL.is_ge)

    # cascade gather: SEL = G[idx : idx+2, :] (6 values per (p, f))
    Gf = G[:].rearrange("p f k t -> p f (k t)")
    SEL = sp.tile([P, FEAT, 6], F32, tag="SEL")
    nc.vector.tensor_copy(out=SEL[:], in_=Gf[:, :, 0:6])
    for k in range(1, NB):
        nc.vector.copy_predicated(
            out=SEL[:],
            mask=PRED[:, :, k - 1:k].to_broadcast([P, FEAT, 6]),
            data=Gf[:, :, 3 * k:3 * k + 6])

    e_lo, eh_lo, d_lo = SEL[:, :, 0], SEL[:, :, 1], SEL[:, :, 2]
    e_hi, eh_hi, d_hi = SEL[:, :, 3], SEL[:, :, 4], SEL[:, :, 5]

    def t64(tag):
        return smp.tile([P, FEAT], F32, tag=tag, name=tag)

    in_w = t64("in_w"); nc.vector.tensor_sub(in_w[:], e_hi, e_lo)
    rw1 = t64("rw1"); nc.vector.reciprocal(rw1[:], in_w[:])
    tnum = t64("tnum"); nc.vector.tensor_sub(tnum[:], xc[:], e_lo)
    th = t64("th"); nc.vector.tensor_mul(th[:], tnum[:], rw1[:])
    th2 = t64("th2")
    nc.scalar.activation(th2[:], th[:], AF.Square, scale=1.0)
    t1mt = t64("t1mt"); nc.vector.tensor_sub(t1mt[:], th[:], th2[:])
    in_h = t64("in_h"); nc.vector.tensor_sub(in_h[:], eh_hi, eh_lo)
    dl = t64("dl"); nc.vector.tensor_mul(dl[:], in_h[:], rw1[:])
    dsum = t64("dsum"); nc.vector.tensor_add(dsum[:], d_lo, d_hi)
    s_ = t64("s_")
    nc.vector.scalar_tensor_tensor(out=s_[:], in0=dl[:], scalar=-2.0,
                                   in1=dsum[:], op0=AL.mult, op1=AL.add)
    m1 = t64("m1"); nc.vector.tensor_mul(m1[:], s_[:], t1mt[:])
    den = t64("den"); nc.vector.tensor_add(den[:], m1[:], dl[:])
    rden = t64("rden"); nc.vector.reciprocal(rden[:], den[:])
    m2 = t64("m2"); nc.vector.tensor_mul(m2[:], dl[:], th2[:])
    m3 = t64("m3"); nc.vector.tensor_mul(m3[:], d_lo, t1mt[:])
    ni = t64("ni"); nc.vector.tensor_add(ni[:], m2[:], m3[:])
    q = t64("q"); nc.vector.tensor_mul(q[:], ni[:], rden[:])
    m4 = t64("m4"); nc.vector.tensor_mul(m4[:], in_h[:], q[:])
    yout = ystage[:, c, :]
    nc.vector.tensor_add(yout, m4[:], eh_lo)

    # outside the interval: y = x
    ax = t64("ax")
    nc.scalar.activation(ax[:], xv, AF.Abs, scale=1.0)
    mo = smp.tile([P, FEAT], U8, tag="mo")
    nc.vector.tensor_scalar(out=mo[:], in0=ax[:], scalar1=TAIL, scalar2=None,
                            op0=AL.is_gt)
    nc.vector.copy_predicated(out=yout, mask=mo[:], data=xv)


def _get_nc(n_tiles, mm_f32r, has_bf):
    key = (n_tiles, mm_f32r, has_bf)
    if key not in _CACHE:
        _CACHE[key] = build(n_tiles, mm_f32r, has_bf)
    return _CACHE[key]


def kernel(x, W0, b0, Wr, br, Wf, bf, n_tiles=BSH // NBT, mm_f32r=True,
           _want_results=False, _trace=False):
    x = np.ascontiguousarray(x, np.float32)
    dev = _host_prep(np.asarray(W0, np.float32), np.asarray(b0, np.float32),
                     np.asarray(Wr, np.float32), np.asarray(br, np.float32),
                     np.asarray(Wf, np.float32), np.asarray(bf, np.float32))
    has_bf = bool(np.any(dev["bf_out"]))
    nc = _get_nc(n_tiles, mm_f32r, has_bf)
    consts = _consts()
    nbsh = n_tiles * NBT

    base = {**{k: np.ascontiguousarray(v, np.float32) for k, v in dev.items()},
            **consts}
    in_maps = []
    for i in range(NCORES):
        m = dict(base)
        m["xs"] = np.ascontiguousarray(x[i * BSH:i * BSH + nbsh])
        in_maps.append(m)
    res = run_bass_kernel_spmd(nc, in_maps, core_ids=list(range(NCORES)),
                               trace=_trace)
    outs = [r["y"] for r in res.results]
    if nbsh == BSH:
        yfull = np.concatenate(outs, axis=0)
    else:
        # dev mode: scatter partial tiles back into a full-shape output
        yfull = np.zeros((B_FULL, FEAT), np.float32)
        for i in range(NCORES):
            yfull[i * BSH:i * BSH + nbsh] = outs[i]
    if _want_results:
        return yfull, res
    return yfull
